# revision 11
# baseline (speedup 1.0000x reference)
"""AdaptiveTokenRefinementModule Trainium2 kernel (8 NeuronCores, 2 batches/core).

v2 of the validated baseline: identical arithmetic (bit-for-bit selection
semantics vs the CPU-jax fp32 oracle), restructured for PE occupancy:
  * x is transposed on the HOST (numpy) and passed as xT [D, S] per batch, so
    the 96-per-batch PE transposes + Scalar psum->sbuf copies disappear.
  * Emission order A0 B0 A1 B1 [C0 || C1]: both batches' selection chains
    (radix-16 threshold search etc.) run interleaved at the end, so their
    DVE->PE round-trip latency is paid once, not twice, and no longer
    head-of-line blocks the next batch's projection/attention matmuls.

Pipeline per batch:
  xT [128,6,S] <- DMA; fp32 matmuls -> qT, kT (1/temp folded into kT on the
  DVE, exactly in fp32); 16 query-chunks of 128 (strided g::16):
  z = qT_g^T @ kT in PSUM -> softmax (DVE reduce_max(negate) -> ScalarE Exp
  with bias=-max, scale=1 -> DVE row-sum -> DVE reciprocal) -> per-key mean
  as scalar_tensor_tensor accumulation + PE ones-matvec -> exact 409-th
  threshold via radix-16 search over positive-float bit patterns -> tie-aware
  top-k mask matching jax.lax.top_k tie-by-index semantics -> prefix-sum
  compaction -> separable one-hot matmuls -> int16 index list in dma_gather's
  16-partition wrapped layout -> gpsimd dma_gather copies exact fp32 rows
  from HBM -> out [409, 768].

Numerical notes (selection must be bit-identical to the CPU-jax oracle):
  * The top-k boundary keys have scores within a few fp32 ulps of 2/2048;
    exactness relies on exp(0)=1.0, correctly-rounded s_q, and fp32 matmuls.
  * z needs full fp32 accuracy (reduced-precision matmul formats measured on
    this hardware: f32r=2cy/row 11-bit, bf16=1cy/row — no split scheme beats
    fp32's 4cy/row at the required accuracy).
  * 1/temp folded into kT (not the ACT scale port, which is not full fp32).
"""
import os
import numpy as np

B, S, D, R = 16, 2048, 768, 384
N_CORES = 8
BPC = B // N_CORES  # batches per core


def _build(red, temp):
    from concourse import bass, bacc, mybir, tile

    F32 = mybir.dt.float32
    I32 = mybir.dt.int32
    I16 = mybir.dt.int16
    AF = mybir.ActivationFunctionType
    ALU = mybir.AluOpType
    AX = mybir.AxisListType
    PSUM = bass.MemorySpace.PSUM

    invT = float(np.float32(1.0) / np.float32(temp))
    inv_s = float(np.float32(1.0) / np.float32(S))  # 1/2048, exact power of 2
    npad = ((red + 127) // 128) * 128              # 512
    nslots = npad // 16                             # 32
    nfull = red // 128                              # 3 full 128-row groups
    ntail = red - nfull * 128                       # 25

    nc = bacc.Bacc(None)
    x_ext = nc.declare_dram_parameter("x", [BPC, S, D], F32, isOutput=False)
    xt_ext = nc.declare_dram_parameter("xT", [BPC, D, S], F32, isOutput=False)
    wqT_ext = nc.declare_dram_parameter("wqT", [D, R], F32, isOutput=False)
    wkT_ext = nc.declare_dram_parameter("wkT", [D, R], F32, isOutput=False)
    bq_ext = nc.declare_dram_parameter("bq", [R], F32, isOutput=False)
    bk_ext = nc.declare_dram_parameter("bk", [R], F32, isOutput=False)
    out_ext = nc.declare_dram_parameter("out", [BPC, red, D], F32, isOutput=True)

    with tile.TileContext(nc) as tc:
        with (
            tc.tile_pool(name="const", bufs=1) as cst,
            tc.tile_pool(name="wts", bufs=1) as wts,
            tc.tile_pool(name="big", bufs=1) as big,
            tc.tile_pool(name="epool", bufs=2) as ep,
            tc.tile_pool(name="small", bufs=1) as sm,
        ):
            # ---------------- constants ----------------
            iota_fp = cst.tile([128, 128], I32)
            nc.gpsimd.iota(iota_fp[:], pattern=[[1, 128]], base=0, channel_multiplier=-1)
            u_strict = cst.tile([128, 128], F32)
            nc.vector.tensor_scalar(u_strict[:], iota_fp[:], 0, None, ALU.is_gt)
            ones_t = cst.tile([128, 1], F32)
            nc.vector.memset(ones_t[:], 1.0)
            ones4 = cst.tile([128, 4], F32)
            nc.vector.memset(ones4[:], 1.0)
            ones16x16 = cst.tile([16, 16], F32)
            nc.vector.memset(ones16x16[:], 1.0)
            lvl_consts = []
            for L in range(8):
                lc = cst.tile([16, 1], I32, name=f"lvlc{L}")
                nc.gpsimd.iota(lc[:], pattern=[[1, 1]], base=0,
                               channel_multiplier=(1 << (4 * L)))
                lvl_consts.append(lc)
            zz16 = cst.tile([128, 16], F32)
            nc.vector.memset(zz16[:], 0.0)
            i16i = cst.tile([128, 16], I32)
            nc.gpsimd.iota(i16i[:], pattern=[[1, 16]], base=0, channel_multiplier=0)
            iota16 = cst.tile([128, 16], F32)
            nc.vector.tensor_copy(iota16[:], i16i[:])
            i32i = cst.tile([128, nslots], I32)
            nc.gpsimd.iota(i32i[:], pattern=[[1, nslots]], base=0, channel_multiplier=0)
            iota32 = cst.tile([128, nslots], F32)
            nc.vector.tensor_copy(iota32[:], i32i[:])
            jci = cst.tile([128, 16], I32)
            nc.gpsimd.iota(jci[:], pattern=[[1, 16]], base=0, channel_multiplier=16)
            jcol_f = cst.tile([128, 16], F32)
            nc.vector.tensor_copy(jcol_f[:], jci[:])
            iwf_i = cst.tile([16, nslots], I32)
            nc.gpsimd.iota(iwf_i[:], pattern=[[16, nslots]], base=0, channel_multiplier=1)
            iota_wf = cst.tile([16, nslots], F32)
            nc.vector.tensor_copy(iota_wf[:], iwf_i[:])
            padmask = cst.tile([16, nslots], F32)
            nc.vector.tensor_scalar(padmask[:], iota_wf[:], float(red), None, ALU.is_lt)
            # fused radix-128 constants: partition p = b*64 + j*4 + c
            pidx = cst.tile([128, 1], I32)
            nc.gpsimd.iota(pidx[:], pattern=[[1, 1]], base=0, channel_multiplier=1)
            jp4 = cst.tile([128, 1], I32)
            nc.vector.tensor_scalar(jp4[:], pidx[:], 2, 15, ALU.logical_shift_right,
                                    ALU.bitwise_and)
            lvl128 = []
            for L in range(8):
                lc = cst.tile([128, 1], I32, name=f"lvl128_{L}")
                nc.vector.tensor_scalar(lc[:], jp4[:], 4 * L, None, ALU.arith_shift_left)
                lvl128.append(lc)
            col128 = cst.tile([128, 128], I32)
            nc.gpsimd.iota(col128[:], pattern=[[1, 128]], base=0, channel_multiplier=0)
            colg_i = cst.tile([128, 128], I32)
            nc.vector.tensor_scalar(colg_i[:], col128[:], 2, None, ALU.logical_shift_right)
            colg = cst.tile([128, 128], F32)
            nc.vector.tensor_copy(colg[:], colg_i[:])
            rowg_i = cst.tile([128, 1], I32)
            nc.vector.tensor_scalar(rowg_i[:], pidx[:], 2, None, ALU.logical_shift_right)
            rowg = cst.tile([128, 1], F32)
            nc.vector.tensor_copy(rowg[:], rowg_i[:])
            Mj = cst.tile([128, 128], F32)
            nc.vector.tensor_scalar(Mj[:], colg[:], rowg[:], None, ALU.is_equal)
            colb_i = cst.tile([128, 128], I32)
            nc.vector.tensor_scalar(colb_i[:], col128[:], 6, None, ALU.logical_shift_right)
            colb = cst.tile([128, 128], F32)
            nc.vector.tensor_copy(colb[:], colb_i[:])
            rowb_i = cst.tile([128, 1], I32)
            nc.vector.tensor_scalar(rowb_i[:], pidx[:], 6, None, ALU.logical_shift_right)
            rowb = cst.tile([128, 1], F32)
            nc.vector.tensor_copy(rowb[:], rowb_i[:])
            Mb = cst.tile([128, 128], F32)
            nc.vector.tensor_scalar(Mb[:], colb[:], rowb[:], 0.25, ALU.is_equal,
                                    ALU.mult)

            # ---------------- weights ----------------
            wq_sb = wts.tile([128, 6, R], F32)
            wk_sb = wts.tile([128, 6, R], F32)
            for d in range(6):
                nc.sync.dma_start(wq_sb[:, d, :],
                                  wqT_ext[d * 128:(d + 1) * 128, :])
            for d in range(6):
                nc.sync.dma_start(wk_sb[:, d, :],
                                  wkT_ext[d * 128:(d + 1) * 128, :])
            bq_sb = wts.tile([128, 3], F32)
            nc.sync.dma_start(bq_sb[:], bq_ext[:].rearrange("(r p) -> p r", p=128))
            bk_sb = wts.tile([128, 3], F32)
            nc.sync.dma_start(bk_sb[:], bk_ext[:].rearrange("(r p) -> p r", p=128))

            qT = {}
            kT = {}
            sc_accs = {}
            s128 = sm.tile([128, 512], F32, tag="s128", name="s128")

            def phaseA(b):
                # xT loaded straight from HBM (host-side transpose), in 4
                # s-chunks so projections can start before the full load.
                xT = big.tile([128, 6, S], F32, tag="xT", name=f"xT{b}")
                for n in range(8):
                    nc.sync.dma_start(
                        xT[:, :, n * 256:(n + 1) * 256],
                        xt_ext[b, :, n * 256:(n + 1) * 256].rearrange(
                            "(c p) s -> p c s", p=128))
                qT[b] = big.tile([128, 3, S], F32, tag="qT", name=f"qT{b}")
                kT[b] = big.tile([128, 3, S], F32, tag="kT", name=f"kT{b}")
                with tc.tile_pool(name=f"psA{b}", bufs=2, space=PSUM) as psA:
                    for dst, w_sb, bias in ((qT[b], wq_sb, bq_sb), (kT[b], wk_sb, bk_sb)):
                        for r in range(3):
                            for n in range(4):
                                pj = psA.tile([128, 512], F32, tag="pj",
                                              name=f"pj{b}_{r}_{n}_{dst.name}")
                                for d in range(6):
                                    nc.tensor.matmul(
                                        pj[:], w_sb[:, d, r * 128:(r + 1) * 128],
                                        xT[:, d, n * 512:(n + 1) * 512],
                                        start=(d == 0), stop=(d == 5))
                                nc.scalar.activation(
                                    dst[:, r, n * 512:(n + 1) * 512], pj[:],
                                    AF.Identity, bias=bias[:, r:r + 1], scale=1.0)
                for r in range(3):
                    nc.vector.tensor_scalar_mul(kT[b][:, r, :], kT[b][:, r, :], invT)

            def phaseB(b):
                with tc.tile_pool(name=f"psB{b}", bufs=2, space=PSUM) as psB:
                    sc_acc = sm.tile([128, S], F32, tag=f"scacc{b}", name=f"scacc{b}")
                    nc.vector.memset(sc_acc[:], 0.0)
                    for g in range(16):
                        z_ps = [psB.tile([128, 512], F32, tag=f"z{n}", name=f"z{b}_{g}_{n}")
                                for n in range(4)]
                        for n in range(4):
                            for kr in range(3):
                                nc.tensor.matmul(
                                    z_ps[n][:], qT[b][:, kr, g::16],
                                    kT[b][:, kr, n * 512:(n + 1) * 512],
                                    start=(kr == 0), stop=(kr == 2))
                        nm = sm.tile([128, 4], F32, tag="nm", bufs=16, name=f"nm{b}_{g}")
                        for n in range(4):
                            nc.vector.tensor_reduce(nm[:, n:n + 1], z_ps[n][:],
                                                    AX.X, ALU.max, negate=True)
                        negm = sm.tile([128, 1], F32, tag="negm", bufs=16, name=f"negm{b}_{g}")
                        nc.vector.tensor_reduce(negm[:], nm[:], AX.X, ALU.min)
                        e_t = ep.tile([128, S], F32, tag="E", name=f"E{b}_{g}")
                        for n in range(4):
                            nc.scalar.activation(e_t[:, n * 512:(n + 1) * 512], z_ps[n][:],
                                                 AF.Exp, bias=negm[:], scale=1.0)
                        s_row = sm.tile([128, 1], F32, tag="srow", bufs=16, name=f"srow{b}_{g}")
                        nc.vector.tensor_reduce(s_row[:], e_t[:], AX.X, ALU.add)
                        w_row = sm.tile([128, 1], F32, tag="wrow", bufs=16, name=f"wrow{b}_{g}")
                        nc.vector.reciprocal(w_row[:], s_row[:])
                        w_s = sm.tile([128, 1], F32, tag="ws", bufs=16, name=f"ws{b}_{g}")
                        nc.vector.tensor_scalar_mul(w_s[:], w_row[:], inv_s)
                        nc.vector.scalar_tensor_tensor(sc_acc[:], e_t[:], w_s[:],
                                                       sc_acc[:], ALU.mult, ALU.add)
                sc_accs[b] = sc_acc

            def fmv_extract(b, pool):
                # each fmv outputs 4 identical rows (ones lhsT with 4 cols):
                # row c of chunk n = the same column sums, bit-identical to a
                # [1,512] matvec, but staged on 4 partitions so downstream
                # DMAs read partitions in parallel (single-partition SBUF
                # reads are slow).
                s4 = sm.tile([4, 512], F32, tag="s4", bufs=2, name=f"s4_{b}")
                for n in range(4):
                    fmv = pool.tile([4, 512], F32, tag="fmv", bufs=2, name=f"fmv{b}_{n}")
                    nc.tensor.matmul(fmv[:], ones4[:],
                                     sc_accs[b][:, n * 512:(n + 1) * 512])
                    stage = sm.tile([4, 512], F32, tag="fmvs", bufs=4,
                                    name=f"fmvs{b}_{n}")
                    nc.vector.tensor_copy(stage[:], fmv[:])
                    nc.sync.dma_start(s4[n:n + 1, :], stage[n:n + 1, :])
                s_t = sm.tile([128, 16], F32, tag=f"st{b}", name=f"st{b}")
                for c in range(4):
                    nc.sync.dma_start(
                        s_t[32 * c:32 * (c + 1), :],
                        s4[c:c + 1, :].rearrange("a (p i) -> a p i", p=32))
                s_ts[b] = s_t
                # spread scores into the fused radix layout: partition
                # b*64 + j*4 + c holds score chunk c (512 wide), j-replicated
                for j in range(16):
                    nc.sync.dma_start(s128[b * 64 + j * 4: b * 64 + (j + 1) * 4, :],
                                      s4[:])

            def radix_fused(psC):
                # exact v* (red-th largest) per batch via radix-16 search on
                # the positive-float bit ordering; both batches in one
                # [128, 512] layout. Counts are small-integer exact.
                t128 = sm.tile([128, 1], I32, tag="t128", bufs=2, name="t128")
                nc.vector.memset(t128[:], 0)
                for L in range(7, -1, -1):
                    cand = sm.tile([128, 1], I32, tag="cand", bufs=2,
                                   name=f"candf_{L}")
                    nc.vector.tensor_tensor(cand[:], t128[:], lvl128[L][:],
                                            ALU.bitwise_or)
                    cmp_t = sm.tile([128, 512], F32, tag="cmpf", bufs=1,
                                    name=f"cmpf_{L}")
                    cnt4 = sm.tile([128, 1], F32, tag="cnt4", bufs=2,
                                   name=f"cnt4_{L}")
                    nc.vector.tensor_scalar(cmp_t[:], s128[:],
                                            cand[:].bitcast(F32), 0.0,
                                            ALU.is_ge, ALU.add,
                                            accum_out=cnt4[:])
                    cnt_ps = psC.tile([128, 1], F32, tag="rc", name=f"cntf_{L}")
                    nc.tensor.matmul(cnt_ps[:], Mj[:], cnt4[:])
                    selj = sm.tile([128, 1], F32, tag="seljf", bufs=2,
                                   name=f"seljf_{L}")
                    nc.vector.tensor_scalar(selj[:], cnt_ps[:], float(red), None,
                                            ALU.is_ge)
                    vm = sm.tile([128, 1], F32, tag="vmf", bufs=2, name=f"vmf_{L}")
                    nc.vector.tensor_scalar(vm[:], cand[:], 0, None, ALU.is_ge)
                    selj2 = sm.tile([128, 1], F32, tag="selj2f", bufs=2,
                                    name=f"selj2f_{L}")
                    nc.vector.tensor_mul(selj2[:], selj[:], vm[:])
                    js_ps = psC.tile([128, 1], F32, tag="rc", name=f"jsf_{L}")
                    nc.tensor.matmul(js_ps[:], Mb[:], selj2[:])
                    jm1 = sm.tile([128, 1], F32, tag="jm1f", bufs=2,
                                  name=f"jm1f_{L}")
                    nc.vector.tensor_scalar(jm1[:], js_ps[:], -1.0, None, ALU.add)
                    jm1_i = sm.tile([128, 1], I32, tag="jm1fi", bufs=2,
                                    name=f"jm1fi_{L}")
                    nc.vector.tensor_copy(jm1_i[:], jm1[:])
                    upd = sm.tile([128, 1], I32, tag="updf", bufs=2,
                                  name=f"updf_{L}")
                    nc.vector.tensor_scalar(upd[:], jm1_i[:], 4 * L, None,
                                            ALU.arith_shift_left)
                    t128n = sm.tile([128, 1], I32, tag="t128", bufs=2,
                                    name=f"t128n_{L}")
                    nc.vector.tensor_tensor(t128n[:], t128[:], upd[:],
                                            ALU.bitwise_or)
                    t128 = t128n
                # stage batch 1's threshold (partition 64) onto partition 0
                tb1s = sm.tile([1, 1], I32, tag="tb1s", name="tb1s")
                nc.sync.dma_start(tb1s[:], t128[64:65, 0:1])
                return t128, tb1s

            def phaseC_gen(b, psC, t128, tb1s):
                # post-threshold selection + gather; yields at cross-engine
                # dependency hops so two batches' chains interleave.
                s_t = s_ts[b]
                t_b = sm.tile([128, 1], F32, tag=f"tb{b}", name=f"tb{b}")
                if b == 0:
                    nc.gpsimd.partition_broadcast(t_b[:], t128[0:1, 0:1].bitcast(F32))
                else:
                    nc.gpsimd.partition_broadcast(t_b[:], tb1s[0:1, 0:1].bitcast(F32))
                yield
                # cnt_gt and m
                sel0 = sm.tile([128, 16], F32, tag=f"sel0{b}", name=f"sel0{b}")
                rs_sel = sm.tile([128, 1], F32, tag=f"rssel{b}", name=f"rssel{b}")
                nc.vector.tensor_scalar(sel0[:], s_t[:], t_b[:], 0.0, ALU.is_gt,
                                        ALU.add, accum_out=rs_sel[:])
                cnt_ps = psC.tile([1, 1], F32, tag=f"c{b}", name=f"cnt{b}")
                nc.tensor.matmul(cnt_ps[:], ones_t[:], rs_sel[:])
                yield
                m_t = sm.tile([1, 1], F32, tag=f"mt{b}", name=f"mt{b}")
                nc.vector.tensor_scalar(m_t[:], cnt_ps[:], -1.0, float(red),
                                        ALU.mult, ALU.add)
                m_b = sm.tile([128, 1], F32, tag=f"mb{b}", name=f"mb{b}")
                nc.gpsimd.partition_broadcast(m_b[:], m_t[:])
                tie = sm.tile([128, 16], F32, tag=f"tie{b}", name=f"tie{b}")
                nc.vector.tensor_scalar(tie[:], s_t[:], t_b[:], None, ALU.is_equal)
                scan_tie = sm.tile([128, 16], F32, tag=f"scant{b}", name=f"scant{b}")
                nc.vector.tensor_tensor_scan(scan_tie[:], tie[:], zz16[:], 0.0,
                                             ALU.add, ALU.add)
                rs_tie = sm.tile([128, 1], F32, tag=f"rstie{b}", name=f"rstie{b}")
                nc.vector.tensor_reduce(rs_tie[:], tie[:], AX.X, ALU.add)
                offt_ps = psC.tile([128, 1], F32, tag=f"c{b}", name=f"offt{b}")
                nc.tensor.matmul(offt_ps[:], u_strict[:], rs_tie[:])
                yield
                off_tie = sm.tile([128, 1], F32, tag=f"offtie{b}", name=f"offtie{b}")
                nc.vector.tensor_copy(off_tie[:], offt_ps[:])
                p_tie = sm.tile([128, 16], F32, tag=f"ptie{b}", name=f"ptie{b}")
                nc.vector.tensor_scalar(p_tie[:], scan_tie[:], off_tie[:], None, ALU.add)

                cond = sm.tile([128, 16], F32, tag=f"cond{b}", name=f"cond{b}")
                nc.vector.tensor_scalar(cond[:], p_tie[:], m_b[:], None, ALU.is_le)
                tsel = sm.tile([128, 16], F32, tag=f"tsel{b}", name=f"tsel{b}")
                nc.vector.tensor_mul(tsel[:], tie[:], cond[:])
                mask = sm.tile([128, 16], F32, tag=f"mask{b}", name=f"mask{b}")
                nc.vector.tensor_add(mask[:], sel0[:], tsel[:])

                scan_m = sm.tile([128, 16], F32, tag=f"scanm{b}", name=f"scanm{b}")
                nc.vector.tensor_tensor_scan(scan_m[:], mask[:], zz16[:], 0.0,
                                             ALU.add, ALU.add)
                rs_m = sm.tile([128, 1], F32, tag=f"rsm{b}", name=f"rsm{b}")
                nc.vector.tensor_reduce(rs_m[:], mask[:], AX.X, ALU.add)
                offm_ps = psC.tile([128, 1], F32, tag=f"c{b}", name=f"offm{b}")
                nc.tensor.matmul(offm_ps[:], u_strict[:], rs_m[:])
                yield
                off_m = sm.tile([128, 1], F32, tag=f"offm{b}", name=f"offmsb{b}")
                nc.vector.tensor_copy(off_m[:], offm_ps[:])
                csum = sm.tile([128, 16], F32, tag=f"csum{b}", name=f"csum{b}")
                nc.vector.tensor_scalar(csum[:], scan_m[:], off_m[:], None, ALU.add)

                # pos0 = mask*(csum+15) - 16  (selected: 0..red-1; unselected: -16)
                t1 = sm.tile([128, 16], F32, tag=f"t1{b}", name=f"t1{b}")
                nc.vector.tensor_scalar(t1[:], csum[:], 15.0, None, ALU.add)
                p1 = sm.tile([128, 16], F32, tag=f"p1{b}", name=f"p1{b}")
                nc.vector.tensor_mul(p1[:], t1[:], mask[:])
                pos0 = sm.tile([128, 16], F32, tag=f"pos0{b}", name=f"pos0{b}")
                nc.vector.tensor_scalar(pos0[:], p1[:], -16.0, None, ALU.add)

                pos_i = sm.tile([128, 16], I32, tag=f"posi{b}", name=f"posi{b}")
                nc.vector.tensor_copy(pos_i[:], pos0[:])
                f_i = sm.tile([128, 16], I32, tag=f"fi{b}", name=f"fi{b}")
                nc.vector.tensor_scalar(f_i[:], pos_i[:], 4, None, ALU.arith_shift_right)
                f16_i = sm.tile([128, 16], I32, tag=f"f16i{b}", name=f"f16i{b}")
                nc.vector.tensor_scalar(f16_i[:], f_i[:], 4, None, ALU.arith_shift_left)
                w_i = sm.tile([128, 16], I32, tag=f"wi{b}", name=f"wi{b}")
                nc.vector.tensor_sub(w_i[:], pos_i[:], f16_i[:])
                f_f = sm.tile([128, 16], F32, tag=f"ff{b}", name=f"ff{b}")
                nc.vector.tensor_copy(f_f[:], f_i[:])
                w_f = sm.tile([128, 16], F32, tag=f"wf{b}", name=f"wf{b}")
                nc.vector.tensor_copy(w_f[:], w_i[:])
                yield

                idx_ps = psC.tile([16, nslots], F32, tag=f"c{b}", name=f"idxps{b}")
                for i in range(16):
                    a_i = sm.tile([128, 16], F32, tag=f"ai{b}", name=f"ai{b}_{i}")
                    nc.vector.tensor_scalar(a_i[:], iota16[:], w_f[:, i:i + 1],
                                            jcol_f[:, i:i + 1], ALU.is_equal, ALU.mult)
                    b_i = sm.tile([128, nslots], F32, tag=f"bi{b}", name=f"bi{b}_{i}")
                    nc.vector.tensor_scalar(b_i[:], iota32[:], f_f[:, i:i + 1], None,
                                            ALU.is_equal)
                    nc.tensor.matmul(idx_ps[:], a_i[:], b_i[:],
                                     start=(i == 0), stop=(i == 15))
                    if i % 6 == 5:
                        yield
                yield

                idx_f = sm.tile([16, nslots], F32, tag=f"idxf{b}", name=f"idxf{b}")
                nc.vector.tensor_scalar(idx_f[:], idx_ps[:], 1.0, None, ALU.add)
                idx_pm = sm.tile([16, nslots], F32, tag=f"idxpm{b}", name=f"idxpm{b}")
                nc.vector.tensor_mul(idx_pm[:], idx_f[:], padmask[:])
                idx_fin = sm.tile([16, nslots], F32, tag=f"idxfin{b}", name=f"idxfin{b}")
                nc.vector.tensor_scalar(idx_fin[:], idx_pm[:], -1.0, None, ALU.add)
                idx16 = sm.tile([16, nslots], I16, tag=f"idx16{b}", name=f"idx16{b}")
                nc.vector.tensor_copy(idx16[:], idx_fin[:])
                yield

                idx128 = sm.tile([128, nslots], I16, tag=f"idx128{b}", name=f"idx128{b}")
                for g in range(8):
                    nc.sync.dma_start(idx128[g * 16:(g + 1) * 16, :], idx16[:])
                yield

                gath = sm.tile([128, npad // 128, D], F32, tag=f"gath{b}", name=f"gath{b}")
                half = npad // 2                      # 256
                hs = half // 16                       # 16 idx slots per half
                hc = half // 128                      # 2 row-groups per half
                nc.gpsimd.dma_gather(gath[:, 0:hc, :], x_ext[b][:],
                                     idx128[:, 0:hs], num_idxs=half,
                                     num_idxs_reg=half, elem_size=D)
                nc.sync.dma_start(
                    out_ext[b, 0:half, :].rearrange("(c p) d -> p c d", c=hc),
                    gath[:, 0:hc, :])
                yield
                nc.gpsimd.dma_gather(gath[:, hc:2 * hc, :], x_ext[b][:],
                                     idx128[:, hs:2 * hs], num_idxs=half,
                                     num_idxs_reg=red - half, elem_size=D)
                if nfull > hc:
                    nc.sync.dma_start(
                        out_ext[b, half:nfull * 128, :].rearrange(
                            "(c p) d -> p c d", c=nfull - hc),
                        gath[:, hc:nfull, :])
                if ntail:
                    nc.sync.dma_start(out_ext[b, nfull * 128:red, :],
                                      gath[0:ntail, nfull, :])

            s_ts = {}
            phaseA(0)
            phaseB(0)
            phaseA(1)
            with tc.tile_pool(name="psF0", bufs=1, space=PSUM) as psF0:
                fmv_extract(0, psF0)
            phaseB(1)

            with tc.tile_pool(name="psC", bufs=2, space=PSUM) as psC:
                fmv_extract(1, psC)
                t128, tb1s = radix_fused(psC)
                gens = [phaseC_gen(b, psC, t128, tb1s) for b in range(BPC)]
                done = [False] * BPC
                while not all(done):
                    for i, g in enumerate(gens):
                        if not done[i]:
                            try:
                                next(g)
                            except StopIteration:
                                done[i] = True

    # schedule audit: for every PSUM tile, its matmuls must appear in the
    # emitted stream (a) start-first and (b) in program order (instruction
    # ids are monotonically assigned at trace time), so fp32 accumulation
    # order is deterministic. The Tile scheduler is nondeterministic; a bad
    # draw is caught here (the caller rebuilds).
    first_mm = {}
    last_id = {}
    ok = True
    for blk in nc.main_func.blocks:
        for ins in blk.instructions:
            if isinstance(ins, mybir.InstMatmult):
                out = ins.outs[0]
                mloc = getattr(out, "memory_location", None)
                name = mloc.name if mloc is not None else getattr(out, "memref", str(out))
                try:
                    iid = int(str(ins.name).split("-")[-1])
                except ValueError:
                    iid = None
                if name not in first_mm:
                    first_mm[name] = ins.start_tensor_calc
                    if not ins.start_tensor_calc:
                        ok = False
                if iid is not None:
                    if name in last_id and iid < last_id[name]:
                        ok = False
                    last_id[name] = iid
    if not ok:
        return None
    nc.compile()
    return nc


_CACHE = {}


def kernel(**inputs):
    from concourse.bass_utils import run_bass_kernel_spmd

    x = np.ascontiguousarray(np.asarray(inputs["x"], dtype=np.float32))
    Wq = np.asarray(inputs["Wq"], dtype=np.float32)
    Wk = np.asarray(inputs["Wk"], dtype=np.float32)
    bq = np.asarray(inputs["bq"], dtype=np.float32)
    bk = np.asarray(inputs["bk"], dtype=np.float32)
    temp = float(np.asarray(inputs["temperature"], dtype=np.float32).reshape(-1)[0])
    num_tokens = int(np.asarray(inputs["num_tokens"]))
    red = int(num_tokens * 0.2)

    key = (red, np.float32(temp).tobytes())
    if key not in _CACHE:
        built = None
        for _attempt in range(4):
            built = _build(red, temp)
            if built is not None:
                break
        assert built is not None, "scheduler audit failed on 4 consecutive builds"
        _CACHE[key] = built
    nc = _CACHE[key]

    wqT = np.ascontiguousarray(Wq.T)  # [D, R]
    wkT = np.ascontiguousarray(Wk.T)
    xT = np.ascontiguousarray(np.swapaxes(x, 1, 2))  # [B, D, S]
    in_maps = [
        {"x": x[i * BPC:(i + 1) * BPC], "xT": xT[i * BPC:(i + 1) * BPC],
         "wqT": wqT, "wkT": wkT, "bq": bq, "bk": bk}
        for i in range(N_CORES)
    ]
    trace = bool(int(os.environ.get("ATRM_TRACE", "0")))
    res = run_bass_kernel_spmd(nc, in_maps, list(range(N_CORES)), trace=trace)
    kernel.last_result = res
    out = np.concatenate([r["out"] for r in res.results], axis=0)
    return out.astype(np.float32)


# revision 13
# speedup vs baseline: 1.2258x; 1.2258x over previous
"""AdaptiveTokenRefinementModule Trainium2 kernel (8 NeuronCores, 2 batches/core).

v2 of the validated baseline: identical arithmetic (bit-for-bit selection
semantics vs the CPU-jax fp32 oracle), restructured for PE occupancy:
  * x is transposed on the HOST (numpy) and passed as xT [D, S] per batch, so
    the 96-per-batch PE transposes + Scalar psum->sbuf copies disappear.
  * Emission order A0 B0 A1 B1 [C0 || C1]: both batches' selection chains
    (radix-16 threshold search etc.) run interleaved at the end, so their
    DVE->PE round-trip latency is paid once, not twice, and no longer
    head-of-line blocks the next batch's projection/attention matmuls.

Pipeline per batch:
  xT [128,6,S] <- DMA; fp32 matmuls -> qT, kT (1/temp folded into kT on the
  DVE, exactly in fp32); 16 query-chunks of 128 (strided g::16):
  z = qT_g^T @ kT in PSUM -> softmax (DVE reduce_max(negate) -> ScalarE Exp
  with bias=-max, scale=1 -> DVE row-sum -> DVE reciprocal) -> per-key mean
  as scalar_tensor_tensor accumulation + PE ones-matvec -> exact 409-th
  threshold via radix-16 search over positive-float bit patterns -> tie-aware
  top-k mask matching jax.lax.top_k tie-by-index semantics -> prefix-sum
  compaction -> separable one-hot matmuls -> int16 index list in dma_gather's
  16-partition wrapped layout -> gpsimd dma_gather copies exact fp32 rows
  from HBM -> out [409, 768].

Numerical notes (selection must be bit-identical to the CPU-jax oracle):
  * The top-k boundary keys have scores within a few fp32 ulps of 2/2048;
    exactness relies on exp(0)=1.0, correctly-rounded s_q, and fp32 matmuls.
  * z needs full fp32 accuracy (reduced-precision matmul formats measured on
    this hardware: f32r=2cy/row 11-bit, bf16=1cy/row — no split scheme beats
    fp32's 4cy/row at the required accuracy).
  * 1/temp folded into kT (not the ACT scale port, which is not full fp32).
"""
import os
import numpy as np

B, S, D, R = 16, 2048, 768, 384
N_CORES = 8
BPC = B // N_CORES  # batches per core


def _build(red, temp):
    from concourse import bass, bacc, mybir, tile

    F32 = mybir.dt.float32
    I32 = mybir.dt.int32
    I16 = mybir.dt.int16
    AF = mybir.ActivationFunctionType
    ALU = mybir.AluOpType
    AX = mybir.AxisListType
    PSUM = bass.MemorySpace.PSUM

    invT = float(np.float32(1.0) / np.float32(temp))
    inv_s = float(np.float32(1.0) / np.float32(S))  # 1/2048, exact power of 2
    npad = ((red + 127) // 128) * 128              # 512
    nslots = npad // 16                             # 32
    nfull = red // 128                              # 3 full 128-row groups
    ntail = red - nfull * 128                       # 25

    nc = bacc.Bacc(None)
    x_ext = nc.declare_dram_parameter("x", [BPC, S, D], F32, isOutput=False)
    xt_ext = nc.declare_dram_parameter("xT", [BPC, D, S], F32, isOutput=False)
    wqT_ext = nc.declare_dram_parameter("wqT", [D, R], F32, isOutput=False)
    wkT_ext = nc.declare_dram_parameter("wkT", [D, R], F32, isOutput=False)
    bq_ext = nc.declare_dram_parameter("bq", [R], F32, isOutput=False)
    bk_ext = nc.declare_dram_parameter("bk", [R], F32, isOutput=False)
    out_ext = nc.declare_dram_parameter("out", [BPC, red, D], F32, isOutput=True)

    with tile.TileContext(nc) as tc:
        with (
            tc.tile_pool(name="const", bufs=1) as cst,
            tc.tile_pool(name="wts", bufs=1) as wts,
            tc.tile_pool(name="big", bufs=1) as big,
            tc.tile_pool(name="epool", bufs=2) as ep,
            tc.tile_pool(name="small", bufs=1) as sm,
        ):
            # ---------------- constants ----------------
            iota_fp = cst.tile([128, 128], I32)
            nc.gpsimd.iota(iota_fp[:], pattern=[[1, 128]], base=0, channel_multiplier=-1)
            u_strict = cst.tile([128, 128], F32)
            nc.vector.tensor_scalar(u_strict[:], iota_fp[:], 0, None, ALU.is_gt)
            ones_t = cst.tile([128, 1], F32)
            nc.vector.memset(ones_t[:], 1.0)
            ones4 = cst.tile([128, 4], F32)
            nc.vector.memset(ones4[:], 1.0)
            ones16x16 = cst.tile([16, 16], F32)
            nc.vector.memset(ones16x16[:], 1.0)
            lvl_consts = []
            for L in range(8):
                lc = cst.tile([16, 1], I32, name=f"lvlc{L}")
                nc.gpsimd.iota(lc[:], pattern=[[1, 1]], base=0,
                               channel_multiplier=(1 << (4 * L)))
                lvl_consts.append(lc)
            zz16 = cst.tile([128, 16], F32)
            nc.vector.memset(zz16[:], 0.0)
            i32i = cst.tile([128, nslots], I32)
            nc.gpsimd.iota(i32i[:], pattern=[[1, nslots]], base=0, channel_multiplier=0)
            iota32 = cst.tile([128, nslots], F32)
            nc.vector.tensor_copy(iota32[:], i32i[:])
            jci = cst.tile([128, 16], I32)
            nc.gpsimd.iota(jci[:], pattern=[[1, 16]], base=0, channel_multiplier=16)
            jcol_f = cst.tile([128, 16], F32)
            nc.vector.tensor_copy(jcol_f[:], jci[:])
            iwf_i = cst.tile([128, nslots], I32)
            nc.gpsimd.iota(iwf_i[:], pattern=[[16, nslots]], base=0, channel_multiplier=1)
            pm16a = cst.tile([128, 1], I32)
            nc.gpsimd.iota(pm16a[:], pattern=[[1, 1]], base=0, channel_multiplier=1)
            pm16b = cst.tile([128, 1], I32)
            nc.vector.tensor_scalar(pm16b[:], pm16a[:], ~15, None, ALU.bitwise_and)
            pm16f = cst.tile([128, 1], F32)
            nc.vector.tensor_copy(pm16f[:], pm16b[:])
            iota_wf = cst.tile([128, nslots], F32)
            nc.vector.tensor_copy(iota_wf[:], iwf_i[:])
            iota_wfm = cst.tile([128, nslots], F32)
            nc.vector.tensor_scalar(iota_wfm[:], iota_wf[:], pm16f[:], None,
                                    ALU.subtract)
            padmask = cst.tile([128, nslots], F32)
            nc.vector.tensor_scalar(padmask[:], iota_wfm[:], float(red), None, ALU.is_lt)
            # fused radix-128 constants. Partition mapping (s16 staging layout):
            # p = b*64 + k*16 + c*4 + a; chunk c = (p>>2)&3;
            # candidate j = 4*((p>>4)&3) + (p&3).
            FP16 = mybir.dt.float16
            pidx = cst.tile([128, 1], I32)
            nc.gpsimd.iota(pidx[:], pattern=[[1, 1]], base=0, channel_multiplier=1)
            jA = cst.tile([128, 1], I32)
            nc.vector.tensor_scalar(jA[:], pidx[:], 2, 12, ALU.logical_shift_right,
                                    ALU.bitwise_and)
            jB = cst.tile([128, 1], I32)
            nc.vector.tensor_scalar(jB[:], pidx[:], 3, None, ALU.bitwise_and)
            jp4 = cst.tile([128, 1], I32)
            nc.vector.tensor_tensor(jp4[:], jA[:], jB[:], ALU.bitwise_or)
            lvl128 = []
            for L in range(8):
                lc = cst.tile([128, 1], I32, name=f"lvl128_{L}")
                nc.vector.tensor_scalar(lc[:], jp4[:], 4 * L, None, ALU.arith_shift_left)
                lvl128.append(lc)
            col128 = cst.tile([128, 128], I32)
            nc.gpsimd.iota(col128[:], pattern=[[1, 128]], base=0, channel_multiplier=0)
            # same (b,j) group <=> p & ~0b1100 equal (chunk bits masked)
            colg_i = cst.tile([128, 128], I32)
            nc.vector.tensor_scalar(colg_i[:], col128[:], ~12, None, ALU.bitwise_and)
            colg = cst.tile([128, 128], F32)
            nc.vector.tensor_copy(colg[:], colg_i[:])
            rowg_i = cst.tile([128, 1], I32)
            nc.vector.tensor_scalar(rowg_i[:], pidx[:], ~12, None, ALU.bitwise_and)
            rowg = cst.tile([128, 1], F32)
            nc.vector.tensor_copy(rowg[:], rowg_i[:])
            Mj = cst.tile([128, 128], F32)
            nc.vector.tensor_scalar(Mj[:], colg[:], rowg[:], None, ALU.is_equal)
            colb_i = cst.tile([128, 128], I32)
            nc.vector.tensor_scalar(colb_i[:], col128[:], 6, None, ALU.logical_shift_right)
            colb = cst.tile([128, 128], F32)
            nc.vector.tensor_copy(colb[:], colb_i[:])
            rowb_i = cst.tile([128, 1], I32)
            nc.vector.tensor_scalar(rowb_i[:], pidx[:], 6, None, ALU.logical_shift_right)
            rowb = cst.tile([128, 1], F32)
            nc.vector.tensor_copy(rowb[:], rowb_i[:])
            Mb32 = cst.tile([128, 128], F32)
            nc.vector.tensor_scalar(Mb32[:], colb[:], rowb[:], 0.25, ALU.is_equal,
                                    ALU.mult)
            Mb = cst.tile([128, 128], FP16)
            nc.vector.tensor_copy(Mb[:], Mb32[:])
            # col%16 pattern for the direct [128, nslots] one-hot index build
            colm_i = cst.tile([128, 128], I32)
            nc.vector.tensor_scalar(colm_i[:], col128[:], 15, None, ALU.bitwise_and)
            colm16 = cst.tile([128, 128], F32)
            nc.vector.tensor_copy(colm16[:], colm_i[:])

            # ---------------- weights ----------------
            wq_sb = wts.tile([128, 6, R], F32)
            wk_sb = wts.tile([128, 6, R], F32)
            for d in range(6):
                nc.sync.dma_start(wq_sb[:, d, :],
                                  wqT_ext[d * 128:(d + 1) * 128, :])
            for d in range(6):
                nc.sync.dma_start(wk_sb[:, d, :],
                                  wkT_ext[d * 128:(d + 1) * 128, :])
            bq_sb = wts.tile([128, 3], F32)
            nc.sync.dma_start(bq_sb[:], bq_ext[:].rearrange("(r p) -> p r", p=128))
            bk_sb = wts.tile([128, 3], F32)
            nc.sync.dma_start(bk_sb[:], bk_ext[:].rearrange("(r p) -> p r", p=128))

            qT = {}
            kT = {}
            sc_accs = {}
            s128 = sm.tile([128, 512], F32, tag="s128", name="s128")

            def phaseA(b):
                # xT loaded straight from HBM (host-side transpose), in 4
                # s-chunks so projections can start before the full load.
                xT = big.tile([128, 6, S], F32, tag="xT", name=f"xT{b}")
                for n in range(8):
                    nc.sync.dma_start(
                        xT[:, :, n * 256:(n + 1) * 256],
                        xt_ext[b, :, n * 256:(n + 1) * 256].rearrange(
                            "(c p) s -> p c s", p=128))
                qT[b] = big.tile([128, 3, S], F32, tag="qT", name=f"qT{b}")
                kT[b] = big.tile([128, 3, S], F32, tag="kT", name=f"kT{b}")
                with tc.tile_pool(name=f"psA{b}", bufs=2, space=PSUM) as psA:
                    for dst, w_sb, bias in ((qT[b], wq_sb, bq_sb), (kT[b], wk_sb, bk_sb)):
                        for r in range(3):
                            for n in range(4):
                                pj = psA.tile([128, 512], F32, tag="pj",
                                              name=f"pj{b}_{r}_{n}_{dst.name}")
                                for d in range(6):
                                    nc.tensor.matmul(
                                        pj[:], w_sb[:, d, r * 128:(r + 1) * 128],
                                        xT[:, d, n * 512:(n + 1) * 512],
                                        start=(d == 0), stop=(d == 5))
                                nc.scalar.activation(
                                    dst[:, r, n * 512:(n + 1) * 512], pj[:],
                                    AF.Identity, bias=bias[:, r:r + 1], scale=1.0)
                for r in range(3):
                    nc.vector.tensor_scalar_mul(kT[b][:, r, :], kT[b][:, r, :], invT)

            def phaseB(b):
                with tc.tile_pool(name=f"psB{b}", bufs=2, space=PSUM) as psB:
                    sc_acc = sm.tile([128, S], F32, tag=f"scacc{b}", name=f"scacc{b}")
                    nc.vector.memset(sc_acc[:], 0.0)
                    for g in range(16):
                        z_ps = [psB.tile([128, 512], F32, tag=f"z{n}", name=f"z{b}_{g}_{n}")
                                for n in range(4)]
                        for n in range(4):
                            for kr in range(3):
                                nc.tensor.matmul(
                                    z_ps[n][:], qT[b][:, kr, g::16],
                                    kT[b][:, kr, n * 512:(n + 1) * 512],
                                    start=(kr == 0), stop=(kr == 2))
                        nm = sm.tile([128, 4], F32, tag="nm", bufs=16, name=f"nm{b}_{g}")
                        for n in range(4):
                            nc.vector.tensor_reduce(nm[:, n:n + 1], z_ps[n][:],
                                                    AX.X, ALU.max, negate=True)
                        negm = sm.tile([128, 1], F32, tag="negm", bufs=16, name=f"negm{b}_{g}")
                        nc.vector.tensor_reduce(negm[:], nm[:], AX.X, ALU.min)
                        e_t = ep.tile([128, S], F32, tag="E", name=f"E{b}_{g}")
                        for n in range(4):
                            nc.scalar.activation(e_t[:, n * 512:(n + 1) * 512], z_ps[n][:],
                                                 AF.Exp, bias=negm[:], scale=1.0)
                        s_row = sm.tile([128, 1], F32, tag="srow", bufs=16, name=f"srow{b}_{g}")
                        nc.vector.tensor_reduce(s_row[:], e_t[:], AX.X, ALU.add)
                        w_row = sm.tile([128, 1], F32, tag="wrow", bufs=16, name=f"wrow{b}_{g}")
                        nc.vector.reciprocal(w_row[:], s_row[:])
                        w_s = sm.tile([128, 1], F32, tag="ws", bufs=16, name=f"ws{b}_{g}")
                        nc.vector.tensor_scalar_mul(w_s[:], w_row[:], inv_s)
                        nc.vector.scalar_tensor_tensor(sc_acc[:], e_t[:], w_s[:],
                                                       sc_acc[:], ALU.mult, ALU.add)
                sc_accs[b] = sc_acc

            def fmv_extract(b, pool):
                # each fmv outputs 4 identical rows (ones lhsT with 4 cols):
                # row c of chunk n = the same column sums, bit-identical to a
                # [1,512] matvec, but staged on multiple partitions so
                # downstream DMAs read partitions in parallel
                # (single-partition SBUF reads are slow).
                s16 = sm.tile([16, 512], F32, tag="s16", bufs=2, name=f"s16_{b}")
                for n in range(4):
                    fmv = pool.tile([4, 512], F32, tag="fmv", bufs=2, name=f"fmv{b}_{n}")
                    nc.tensor.matmul(fmv[:], ones4[:],
                                     sc_accs[b][:, n * 512:(n + 1) * 512])
                    stage = sm.tile([4, 512], F32, tag="fmvs", bufs=4,
                                    name=f"fmvs{b}_{n}")
                    nc.vector.tensor_copy(stage[:], fmv[:])
                    nc.sync.dma_start(s16[4 * n:4 * (n + 1), :], stage[:])
                s_t = sm.tile([128, 16], F32, tag=f"st{b}", name=f"st{b}")
                for c in range(4):
                    nc.gpsimd.dma_start(
                        s_t[32 * c:32 * (c + 1), :],
                        s16[4 * c:4 * c + 1, :].rearrange("a (p i) -> a p i", p=32))
                s_ts[b] = s_t
                # spread into the radix layout: 4 quarter-copies per half
                for k in range(4):
                    nc.sync.dma_start(s128[b * 64 + 16 * k: b * 64 + 16 * (k + 1), :],
                                      s16[:])

            def radix_fused(psC):
                # exact v* (red-th largest) per batch via radix-16 search on
                # the positive-float bit ordering; both batches in one
                # [128, 512] layout. Counts are small-integer exact.
                t128 = sm.tile([128, 1], I32, tag="t128", bufs=2, name="t128")
                nc.vector.memset(t128[:], 0)
                for L in range(7, -1, -1):
                    cand = sm.tile([128, 1], I32, tag="cand", bufs=2,
                                   name=f"candf_{L}")
                    nc.vector.tensor_tensor(cand[:], t128[:], lvl128[L][:],
                                            ALU.bitwise_or)
                    cmp_t = sm.tile([128, 512], F32, tag="cmpf", bufs=1,
                                    name=f"cmpf_{L}")
                    cnt4 = sm.tile([128, 1], F32, tag="cnt4", bufs=2,
                                   name=f"cnt4_{L}")
                    nc.vector.tensor_scalar(cmp_t[:], s128[:],
                                            cand[:].bitcast(F32), 0.0,
                                            ALU.is_ge, ALU.add,
                                            accum_out=cnt4[:])
                    vm = sm.tile([128, 1], mybir.dt.float16, tag="vmf", bufs=2,
                                 name=f"vmf_{L}")
                    nc.vector.tensor_scalar(vm[:], cand[:], 0, None, ALU.is_ge)
                    cnt_ps = psC.tile([128, 1], F32, tag="rc", name=f"cntf_{L}")
                    nc.tensor.matmul(cnt_ps[:], Mj[:], cnt4[:])
                    selj2 = sm.tile([128, 1], mybir.dt.float16, tag="selj2f", bufs=2,
                                    name=f"selj2f_{L}")
                    nc.vector.scalar_tensor_tensor(selj2[:], cnt_ps[:], float(red),
                                                   vm[:], ALU.is_ge, ALU.mult)
                    js_ps = psC.tile([128, 1], F32, tag="rc", name=f"jsf_{L}")
                    nc.tensor.matmul(js_ps[:], Mb[:], selj2[:])
                    jm1_i = sm.tile([128, 1], I32, tag="jm1fi", bufs=2,
                                    name=f"jm1fi_{L}")
                    nc.vector.tensor_scalar(jm1_i[:], js_ps[:], -1.0, None, ALU.add)
                    upd = sm.tile([128, 1], I32, tag="updf", bufs=2,
                                  name=f"updf_{L}")
                    nc.vector.tensor_scalar(upd[:], jm1_i[:], 4 * L, None,
                                            ALU.arith_shift_left)
                    t128n = sm.tile([128, 1], I32, tag="t128", bufs=2,
                                    name=f"t128n_{L}")
                    nc.vector.tensor_tensor(t128n[:], t128[:], upd[:],
                                            ALU.bitwise_or)
                    t128 = t128n
                # stage batch 1's threshold (partition 64) onto partition 0
                tb1s = sm.tile([1, 1], I32, tag="tb1s", name="tb1s")
                nc.sync.dma_start(tb1s[:], t128[64:65, 0:1])
                return t128, tb1s

            def phaseC_gen(b, psC, t128, tb1s):
                # post-threshold selection + gather; yields at cross-engine
                # dependency hops so two batches' chains interleave.
                s_t = s_ts[b]
                t_b = sm.tile([128, 1], F32, tag=f"tb{b}", name=f"tb{b}")
                if b == 0:
                    nc.gpsimd.partition_broadcast(t_b[:], t128[0:1, 0:1].bitcast(F32))
                else:
                    nc.gpsimd.partition_broadcast(t_b[:], tb1s[0:1, 0:1].bitcast(F32))
                yield
                # cnt_gt and m
                sel0 = sm.tile([128, 16], F32, tag=f"sel0{b}", name=f"sel0{b}")
                rs_sel = sm.tile([128, 1], F32, tag=f"rssel{b}", name=f"rssel{b}")
                nc.vector.tensor_scalar(sel0[:], s_t[:], t_b[:], 0.0, ALU.is_gt,
                                        ALU.add, accum_out=rs_sel[:])
                cnt_ps = psC.tile([1, 1], F32, tag=f"c{b}", name=f"cnt{b}")
                nc.tensor.matmul(cnt_ps[:], ones_t[:], rs_sel[:])
                yield
                m_t = sm.tile([1, 1], F32, tag=f"mt{b}", name=f"mt{b}")
                nc.vector.tensor_scalar(m_t[:], cnt_ps[:], -1.0, float(red),
                                        ALU.mult, ALU.add)
                m_b = sm.tile([128, 1], F32, tag=f"mb{b}", name=f"mb{b}")
                nc.gpsimd.partition_broadcast(m_b[:], m_t[:])
                tie = sm.tile([128, 16], F32, tag=f"tie{b}", name=f"tie{b}")
                nc.vector.tensor_scalar(tie[:], s_t[:], t_b[:], None, ALU.is_equal)
                scan_tie = sm.tile([128, 16], F32, tag=f"scant{b}", name=f"scant{b}")
                nc.vector.tensor_tensor_scan(scan_tie[:], tie[:], zz16[:], 0.0,
                                             ALU.add, ALU.add)
                rs_tie = sm.tile([128, 1], F32, tag=f"rstie{b}", name=f"rstie{b}")
                nc.vector.tensor_reduce(rs_tie[:], tie[:], AX.X, ALU.add)
                offt_ps = psC.tile([128, 1], F32, tag=f"c{b}", name=f"offt{b}")
                nc.tensor.matmul(offt_ps[:], u_strict[:], rs_tie[:])
                yield
                off_tie = sm.tile([128, 1], F32, tag=f"offtie{b}", name=f"offtie{b}")
                nc.vector.tensor_copy(off_tie[:], offt_ps[:])
                p_tie = sm.tile([128, 16], F32, tag=f"ptie{b}", name=f"ptie{b}")
                nc.vector.tensor_scalar(p_tie[:], scan_tie[:], off_tie[:], None, ALU.add)

                cond = sm.tile([128, 16], F32, tag=f"cond{b}", name=f"cond{b}")
                nc.vector.tensor_scalar(cond[:], p_tie[:], m_b[:], None, ALU.is_le)
                tsel = sm.tile([128, 16], F32, tag=f"tsel{b}", name=f"tsel{b}")
                nc.vector.tensor_mul(tsel[:], tie[:], cond[:])
                mask = sm.tile([128, 16], F32, tag=f"mask{b}", name=f"mask{b}")
                nc.vector.tensor_add(mask[:], sel0[:], tsel[:])

                scan_m = sm.tile([128, 16], F32, tag=f"scanm{b}", name=f"scanm{b}")
                nc.vector.tensor_tensor_scan(scan_m[:], mask[:], zz16[:], 0.0,
                                             ALU.add, ALU.add)
                rs_m = sm.tile([128, 1], F32, tag=f"rsm{b}", name=f"rsm{b}")
                nc.vector.tensor_reduce(rs_m[:], mask[:], AX.X, ALU.add)
                offm_ps = psC.tile([128, 1], F32, tag=f"c{b}", name=f"offm{b}")
                nc.tensor.matmul(offm_ps[:], u_strict[:], rs_m[:])
                yield
                off_m = sm.tile([128, 1], F32, tag=f"offm{b}", name=f"offmsb{b}")
                nc.vector.tensor_copy(off_m[:], offm_ps[:])
                csum = sm.tile([128, 16], F32, tag=f"csum{b}", name=f"csum{b}")
                nc.vector.tensor_scalar(csum[:], scan_m[:], off_m[:], None, ALU.add)

                # pos0 = mask*(csum+15) - 16  (selected: 0..red-1; unselected: -16)
                t1 = sm.tile([128, 16], F32, tag=f"t1{b}", name=f"t1{b}")
                nc.vector.tensor_scalar(t1[:], csum[:], 15.0, None, ALU.add)
                p1 = sm.tile([128, 16], F32, tag=f"p1{b}", name=f"p1{b}")
                nc.vector.tensor_mul(p1[:], t1[:], mask[:])
                pos0 = sm.tile([128, 16], F32, tag=f"pos0{b}", name=f"pos0{b}")
                nc.vector.tensor_scalar(pos0[:], p1[:], -16.0, None, ALU.add)

                pos_i = sm.tile([128, 16], I32, tag=f"posi{b}", name=f"posi{b}")
                nc.vector.tensor_copy(pos_i[:], pos0[:])
                f_i = sm.tile([128, 16], I32, tag=f"fi{b}", name=f"fi{b}")
                nc.vector.tensor_scalar(f_i[:], pos_i[:], 4, None, ALU.arith_shift_right)
                f16_i = sm.tile([128, 16], I32, tag=f"f16i{b}", name=f"f16i{b}")
                nc.vector.tensor_scalar(f16_i[:], f_i[:], 4, None, ALU.arith_shift_left)
                w_i = sm.tile([128, 16], I32, tag=f"wi{b}", name=f"wi{b}")
                nc.vector.tensor_sub(w_i[:], pos_i[:], f16_i[:])
                f_f = sm.tile([128, 16], F32, tag=f"ff{b}", name=f"ff{b}")
                nc.vector.tensor_copy(f_f[:], f_i[:])
                w_f = sm.tile([128, 16], F32, tag=f"wf{b}", name=f"wf{b}")
                nc.vector.tensor_copy(w_f[:], w_i[:])
                yield

                idx_ps = psC.tile([128, nslots], F32, tag=f"c{b}", name=f"idxps{b}")
                for i in range(16):
                    a_i = sm.tile([128, 128], F32, tag=f"ai{b}", name=f"ai{b}_{i}")
                    nc.vector.tensor_scalar(a_i[:], colm16[:], w_f[:, i:i + 1],
                                            jcol_f[:, i:i + 1], ALU.is_equal, ALU.mult)
                    b_i = sm.tile([128, nslots], F32, tag=f"bi{b}", name=f"bi{b}_{i}")
                    nc.vector.tensor_scalar(b_i[:], iota32[:], f_f[:, i:i + 1], None,
                                            ALU.is_equal)
                    nc.tensor.matmul(idx_ps[:], a_i[:], b_i[:],
                                     start=(i == 0), stop=(i == 15))
                    if i % 6 == 5:
                        yield
                yield

                idx_f = sm.tile([128, nslots], F32, tag=f"idxf{b}", name=f"idxf{b}")
                nc.vector.tensor_scalar(idx_f[:], idx_ps[:], 1.0, None, ALU.add)
                idx_pm = sm.tile([128, nslots], F32, tag=f"idxpm{b}", name=f"idxpm{b}")
                nc.vector.tensor_mul(idx_pm[:], idx_f[:], padmask[:])
                idx_fin = sm.tile([128, nslots], F32, tag=f"idxfin{b}", name=f"idxfin{b}")
                nc.vector.tensor_scalar(idx_fin[:], idx_pm[:], -1.0, None, ALU.add)
                idx128 = sm.tile([128, nslots], I16, tag=f"idx128{b}", name=f"idx128{b}")
                nc.vector.tensor_copy(idx128[:], idx_fin[:])
                yield

                gath = sm.tile([128, npad // 128, D], F32, tag=f"gath{b}", name=f"gath{b}")
                half = npad // 2                      # 256
                hs = half // 16                       # 16 idx slots per half
                hc = half // 128                      # 2 row-groups per half
                nc.gpsimd.dma_gather(gath[:, 0:hc, :], x_ext[b][:],
                                     idx128[:, 0:hs], num_idxs=half,
                                     num_idxs_reg=half, elem_size=D)
                nc.sync.dma_start(
                    out_ext[b, 0:half, :].rearrange("(c p) d -> p c d", c=hc),
                    gath[:, 0:hc, :])
                yield
                nc.gpsimd.dma_gather(gath[:, hc:2 * hc, :], x_ext[b][:],
                                     idx128[:, hs:2 * hs], num_idxs=half,
                                     num_idxs_reg=red - half, elem_size=D)
                if nfull > hc:
                    nc.sync.dma_start(
                        out_ext[b, half:nfull * 128, :].rearrange(
                            "(c p) d -> p c d", c=nfull - hc),
                        gath[:, hc:nfull, :])
                if ntail:
                    nc.sync.dma_start(out_ext[b, nfull * 128:red, :],
                                      gath[0:ntail, nfull, :])

            s_ts = {}
            phaseA(0)
            phaseB(0)
            phaseA(1)
            with tc.tile_pool(name="psF0", bufs=1, space=PSUM) as psF0:
                fmv_extract(0, psF0)
            phaseB(1)

            with tc.tile_pool(name="psC", bufs=2, space=PSUM) as psC:
                fmv_extract(1, psC)
                t128, tb1s = radix_fused(psC)
                gens = [phaseC_gen(b, psC, t128, tb1s) for b in range(BPC)]
                done = [False] * BPC
                while not all(done):
                    for i, g in enumerate(gens):
                        if not done[i]:
                            try:
                                next(g)
                            except StopIteration:
                                done[i] = True

    # schedule audit: for every PSUM tile, its matmuls must appear in the
    # emitted stream (a) start-first and (b) in program order (instruction
    # ids are monotonically assigned at trace time), so fp32 accumulation
    # order is deterministic. The Tile scheduler is nondeterministic; a bad
    # draw is caught here (the caller rebuilds).
    first_mm = {}
    last_id = {}
    ok = True
    for blk in nc.main_func.blocks:
        for ins in blk.instructions:
            if isinstance(ins, mybir.InstMatmult):
                out = ins.outs[0]
                mloc = getattr(out, "memory_location", None)
                name = mloc.name if mloc is not None else getattr(out, "memref", str(out))
                try:
                    iid = int(str(ins.name).split("-")[-1])
                except ValueError:
                    iid = None
                if name not in first_mm:
                    first_mm[name] = ins.start_tensor_calc
                    if not ins.start_tensor_calc:
                        ok = False
                if iid is not None:
                    if name in last_id and iid < last_id[name]:
                        ok = False
                    last_id[name] = iid
    if not ok:
        return None
    nc.compile()
    return nc


_CACHE = {}


def kernel(**inputs):
    from concourse.bass_utils import run_bass_kernel_spmd

    x = np.ascontiguousarray(np.asarray(inputs["x"], dtype=np.float32))
    Wq = np.asarray(inputs["Wq"], dtype=np.float32)
    Wk = np.asarray(inputs["Wk"], dtype=np.float32)
    bq = np.asarray(inputs["bq"], dtype=np.float32)
    bk = np.asarray(inputs["bk"], dtype=np.float32)
    temp = float(np.asarray(inputs["temperature"], dtype=np.float32).reshape(-1)[0])
    num_tokens = int(np.asarray(inputs["num_tokens"]))
    red = int(num_tokens * 0.2)

    key = (red, np.float32(temp).tobytes())
    if key not in _CACHE:
        built = None
        for _attempt in range(4):
            built = _build(red, temp)
            if built is not None:
                break
        assert built is not None, "scheduler audit failed on 4 consecutive builds"
        _CACHE[key] = built
    nc = _CACHE[key]

    wqT = np.ascontiguousarray(Wq.T)  # [D, R]
    wkT = np.ascontiguousarray(Wk.T)
    xT = np.ascontiguousarray(np.swapaxes(x, 1, 2))  # [B, D, S]
    in_maps = [
        {"x": x[i * BPC:(i + 1) * BPC], "xT": xT[i * BPC:(i + 1) * BPC],
         "wqT": wqT, "wkT": wkT, "bq": bq, "bk": bk}
        for i in range(N_CORES)
    ]
    trace = bool(int(os.environ.get("ATRM_TRACE", "0")))
    res = run_bass_kernel_spmd(nc, in_maps, list(range(N_CORES)), trace=trace)
    kernel.last_result = res
    out = np.concatenate([r["out"] for r in res.results], axis=0)
    return out.astype(np.float32)


# revision 14
# speedup vs baseline: 1.2375x; 1.0096x over previous
"""AdaptiveTokenRefinementModule Trainium2 kernel (8 NeuronCores, 2 batches/core).

v2 of the validated baseline: identical arithmetic (bit-for-bit selection
semantics vs the CPU-jax fp32 oracle), restructured for PE occupancy:
  * x is transposed on the HOST (numpy) and passed as xT [D, S] per batch, so
    the 96-per-batch PE transposes + Scalar psum->sbuf copies disappear.
  * Emission order A0 B0 A1 B1 [C0 || C1]: both batches' selection chains
    (radix-16 threshold search etc.) run interleaved at the end, so their
    DVE->PE round-trip latency is paid once, not twice, and no longer
    head-of-line blocks the next batch's projection/attention matmuls.

Pipeline per batch:
  xT [128,6,S] <- DMA; fp32 matmuls -> qT, kT (1/temp folded into kT on the
  DVE, exactly in fp32); 16 query-chunks of 128 (strided g::16):
  z = qT_g^T @ kT in PSUM -> softmax (DVE reduce_max(negate) -> ScalarE Exp
  with bias=-max, scale=1 -> DVE row-sum -> DVE reciprocal) -> per-key mean
  as scalar_tensor_tensor accumulation + PE ones-matvec -> exact 409-th
  threshold via radix-16 search over positive-float bit patterns -> tie-aware
  top-k mask matching jax.lax.top_k tie-by-index semantics -> prefix-sum
  compaction -> separable one-hot matmuls -> int16 index list in dma_gather's
  16-partition wrapped layout -> gpsimd dma_gather copies exact fp32 rows
  from HBM -> out [409, 768].

Numerical notes (selection must be bit-identical to the CPU-jax oracle):
  * The top-k boundary keys have scores within a few fp32 ulps of 2/2048;
    exactness relies on exp(0)=1.0, correctly-rounded s_q, and fp32 matmuls.
  * z needs full fp32 accuracy (reduced-precision matmul formats measured on
    this hardware: f32r=2cy/row 11-bit, bf16=1cy/row — no split scheme beats
    fp32's 4cy/row at the required accuracy).
  * 1/temp folded into kT (not the ACT scale port, which is not full fp32).
"""
import os
import numpy as np

B, S, D, R = 16, 2048, 768, 384
N_CORES = 8
BPC = B // N_CORES  # batches per core


def _build(red, temp):
    from concourse import bass, bacc, mybir, tile

    F32 = mybir.dt.float32
    I32 = mybir.dt.int32
    I16 = mybir.dt.int16
    AF = mybir.ActivationFunctionType
    ALU = mybir.AluOpType
    AX = mybir.AxisListType
    PSUM = bass.MemorySpace.PSUM

    invT = float(np.float32(1.0) / np.float32(temp))
    inv_s = float(np.float32(1.0) / np.float32(S))  # 1/2048, exact power of 2
    npad = ((red + 127) // 128) * 128              # 512
    nslots = npad // 16                             # 32
    nfull = red // 128                              # 3 full 128-row groups
    ntail = red - nfull * 128                       # 25

    nc = bacc.Bacc(None)
    x_ext = nc.declare_dram_parameter("x", [BPC, S, D], F32, isOutput=False)
    xt_ext = nc.declare_dram_parameter("xT", [BPC, D, S], F32, isOutput=False)
    wqT_ext = nc.declare_dram_parameter("wqT", [D, R], F32, isOutput=False)
    wkT_ext = nc.declare_dram_parameter("wkT", [D, R], F32, isOutput=False)
    bq_ext = nc.declare_dram_parameter("bq", [R], F32, isOutput=False)
    bk_ext = nc.declare_dram_parameter("bk", [R], F32, isOutput=False)
    out_ext = nc.declare_dram_parameter("out", [BPC, red, D], F32, isOutput=True)

    with tile.TileContext(nc) as tc:
        with (
            tc.tile_pool(name="const", bufs=1) as cst,
            tc.tile_pool(name="wts", bufs=1) as wts,
            tc.tile_pool(name="big", bufs=1) as big,
            tc.tile_pool(name="epool", bufs=2) as ep,
            tc.tile_pool(name="small", bufs=1) as sm,
        ):
            # ---------------- constants ----------------
            iota_fp = cst.tile([128, 128], I32)
            nc.gpsimd.iota(iota_fp[:], pattern=[[1, 128]], base=0, channel_multiplier=-1)
            u_strict = cst.tile([128, 128], F32)
            nc.vector.tensor_scalar(u_strict[:], iota_fp[:], 0, None, ALU.is_gt)
            ones_t = cst.tile([128, 1], F32)
            nc.vector.memset(ones_t[:], 1.0)
            ones4 = cst.tile([128, 4], F32)
            nc.vector.memset(ones4[:], 1.0)
            ones16x16 = cst.tile([16, 16], F32)
            nc.vector.memset(ones16x16[:], 1.0)
            lvl_consts = []
            for L in range(8):
                lc = cst.tile([16, 1], I32, name=f"lvlc{L}")
                nc.gpsimd.iota(lc[:], pattern=[[1, 1]], base=0,
                               channel_multiplier=(1 << (4 * L)))
                lvl_consts.append(lc)
            zz16 = cst.tile([128, 16], F32)
            nc.vector.memset(zz16[:], 0.0)
            i32i = cst.tile([128, nslots], I32)
            nc.gpsimd.iota(i32i[:], pattern=[[1, nslots]], base=0, channel_multiplier=0)
            iota32 = cst.tile([128, nslots], F32)
            nc.vector.tensor_copy(iota32[:], i32i[:])
            jci = cst.tile([128, 16], I32)
            nc.gpsimd.iota(jci[:], pattern=[[1, 16]], base=0, channel_multiplier=16)
            jcol_f = cst.tile([128, 16], F32)
            nc.vector.tensor_copy(jcol_f[:], jci[:])
            iwf_i = cst.tile([128, nslots], I32)
            nc.gpsimd.iota(iwf_i[:], pattern=[[16, nslots]], base=0, channel_multiplier=1)
            pm16a = cst.tile([128, 1], I32)
            nc.gpsimd.iota(pm16a[:], pattern=[[1, 1]], base=0, channel_multiplier=1)
            pm16b = cst.tile([128, 1], I32)
            nc.vector.tensor_scalar(pm16b[:], pm16a[:], ~15, None, ALU.bitwise_and)
            pm16f = cst.tile([128, 1], F32)
            nc.vector.tensor_copy(pm16f[:], pm16b[:])
            iota_wf = cst.tile([128, nslots], F32)
            nc.vector.tensor_copy(iota_wf[:], iwf_i[:])
            iota_wfm = cst.tile([128, nslots], F32)
            nc.vector.tensor_scalar(iota_wfm[:], iota_wf[:], pm16f[:], None,
                                    ALU.subtract)
            padmask = cst.tile([128, nslots], F32)
            nc.vector.tensor_scalar(padmask[:], iota_wfm[:], float(red), None, ALU.is_lt)
            # fused radix-128 constants. Partition mapping (s16 staging layout):
            # p = b*64 + k*16 + c*4 + a; chunk c = (p>>2)&3;
            # candidate j = 4*((p>>4)&3) + (p&3).
            FP16 = mybir.dt.float16
            pidx = cst.tile([128, 1], I32)
            nc.gpsimd.iota(pidx[:], pattern=[[1, 1]], base=0, channel_multiplier=1)
            jA = cst.tile([128, 1], I32)
            nc.vector.tensor_scalar(jA[:], pidx[:], 2, 12, ALU.logical_shift_right,
                                    ALU.bitwise_and)
            jB = cst.tile([128, 1], I32)
            nc.vector.tensor_scalar(jB[:], pidx[:], 3, None, ALU.bitwise_and)
            jp4 = cst.tile([128, 1], I32)
            nc.vector.tensor_tensor(jp4[:], jA[:], jB[:], ALU.bitwise_or)
            lvl128 = []
            for L in range(8):
                lc = cst.tile([128, 1], I32, name=f"lvl128_{L}")
                nc.vector.tensor_scalar(lc[:], jp4[:], 4 * L, None, ALU.arith_shift_left)
                lvl128.append(lc)
            col128 = cst.tile([128, 128], I32)
            nc.gpsimd.iota(col128[:], pattern=[[1, 128]], base=0, channel_multiplier=0)
            # same (b,j) group <=> p & ~0b1100 equal (chunk bits masked)
            colg_i = cst.tile([128, 128], I32)
            nc.vector.tensor_scalar(colg_i[:], col128[:], ~12, None, ALU.bitwise_and)
            colg = cst.tile([128, 128], F32)
            nc.vector.tensor_copy(colg[:], colg_i[:])
            rowg_i = cst.tile([128, 1], I32)
            nc.vector.tensor_scalar(rowg_i[:], pidx[:], ~12, None, ALU.bitwise_and)
            rowg = cst.tile([128, 1], F32)
            nc.vector.tensor_copy(rowg[:], rowg_i[:])
            Mj32 = cst.tile([128, 128], F32)
            nc.vector.tensor_scalar(Mj32[:], colg[:], rowg[:], None, ALU.is_equal)
            Mj = cst.tile([128, 128], FP16)
            nc.vector.tensor_copy(Mj[:], Mj32[:])
            colb_i = cst.tile([128, 128], I32)
            nc.vector.tensor_scalar(colb_i[:], col128[:], 6, None, ALU.logical_shift_right)
            colb = cst.tile([128, 128], F32)
            nc.vector.tensor_copy(colb[:], colb_i[:])
            rowb_i = cst.tile([128, 1], I32)
            nc.vector.tensor_scalar(rowb_i[:], pidx[:], 6, None, ALU.logical_shift_right)
            rowb = cst.tile([128, 1], F32)
            nc.vector.tensor_copy(rowb[:], rowb_i[:])
            Mb32 = cst.tile([128, 128], F32)
            nc.vector.tensor_scalar(Mb32[:], colb[:], rowb[:], 0.25, ALU.is_equal,
                                    ALU.mult)
            Mb = cst.tile([128, 128], FP16)
            nc.vector.tensor_copy(Mb[:], Mb32[:])
            # col%16 pattern for the direct [128, nslots] one-hot index build
            colm_i = cst.tile([128, 128], I32)
            nc.vector.tensor_scalar(colm_i[:], col128[:], 15, None, ALU.bitwise_and)
            colm16 = cst.tile([128, 128], F32)
            nc.vector.tensor_copy(colm16[:], colm_i[:])

            # ---------------- weights ----------------
            wq_sb = wts.tile([128, 6, R], F32)
            wk_sb = wts.tile([128, 6, R], F32)
            for d in range(6):
                nc.sync.dma_start(wq_sb[:, d, :],
                                  wqT_ext[d * 128:(d + 1) * 128, :])
            for d in range(6):
                nc.sync.dma_start(wk_sb[:, d, :],
                                  wkT_ext[d * 128:(d + 1) * 128, :])
            bq_sb = wts.tile([128, 3], F32)
            nc.sync.dma_start(bq_sb[:], bq_ext[:].rearrange("(r p) -> p r", p=128))
            bk_sb = wts.tile([128, 3], F32)
            nc.sync.dma_start(bk_sb[:], bk_ext[:].rearrange("(r p) -> p r", p=128))

            qT = {}
            kT = {}
            sc_accs = {}
            s128 = sm.tile([128, 512], F32, tag="s128", name="s128")

            def phaseA(b):
                # xT loaded straight from HBM (host-side transpose), in 4
                # s-chunks so projections can start before the full load.
                xT = big.tile([128, 6, S], F32, tag="xT", name=f"xT{b}")
                for n in range(8):
                    nc.sync.dma_start(
                        xT[:, :, n * 256:(n + 1) * 256],
                        xt_ext[b, :, n * 256:(n + 1) * 256].rearrange(
                            "(c p) s -> p c s", p=128))
                qT[b] = big.tile([128, 3, S], F32, tag="qT", name=f"qT{b}")
                kT[b] = big.tile([128, 3, S], F32, tag="kT", name=f"kT{b}")
                with tc.tile_pool(name=f"psA{b}", bufs=2, space=PSUM) as psA:
                    for dst, w_sb, bias in ((qT[b], wq_sb, bq_sb), (kT[b], wk_sb, bk_sb)):
                        for r in range(3):
                            for n in range(4):
                                pj = psA.tile([128, 512], F32, tag="pj",
                                              name=f"pj{b}_{r}_{n}_{dst.name}")
                                for d in range(6):
                                    nc.tensor.matmul(
                                        pj[:], w_sb[:, d, r * 128:(r + 1) * 128],
                                        xT[:, d, n * 512:(n + 1) * 512],
                                        start=(d == 0), stop=(d == 5))
                                nc.scalar.activation(
                                    dst[:, r, n * 512:(n + 1) * 512], pj[:],
                                    AF.Identity, bias=bias[:, r:r + 1], scale=1.0)
                for r in range(3):
                    nc.vector.tensor_scalar_mul(kT[b][:, r, :], kT[b][:, r, :], invT)

            def phaseB(b):
                with tc.tile_pool(name=f"psB{b}", bufs=2, space=PSUM) as psB:
                    sc_acc = sm.tile([128, S], F32, tag=f"scacc{b}", name=f"scacc{b}")
                    nc.vector.memset(sc_acc[:], 0.0)
                    for g in range(16):
                        z_ps = [psB.tile([128, 512], F32, tag=f"z{n}", name=f"z{b}_{g}_{n}")
                                for n in range(4)]
                        for n in range(4):
                            for kr in range(3):
                                nc.tensor.matmul(
                                    z_ps[n][:], qT[b][:, kr, g::16],
                                    kT[b][:, kr, n * 512:(n + 1) * 512],
                                    start=(kr == 0), stop=(kr == 2))
                        nm = sm.tile([128, 4], F32, tag="nm", bufs=16, name=f"nm{b}_{g}")
                        for n in range(4):
                            nc.vector.tensor_reduce(nm[:, n:n + 1], z_ps[n][:],
                                                    AX.X, ALU.max, negate=True)
                        negm = sm.tile([128, 1], F32, tag="negm", bufs=16, name=f"negm{b}_{g}")
                        nc.vector.tensor_reduce(negm[:], nm[:], AX.X, ALU.min)
                        e_t = ep.tile([128, S], F32, tag="E", name=f"E{b}_{g}")
                        for n in range(4):
                            nc.scalar.activation(e_t[:, n * 512:(n + 1) * 512], z_ps[n][:],
                                                 AF.Exp, bias=negm[:], scale=1.0)
                        s_row = sm.tile([128, 1], F32, tag="srow", bufs=16, name=f"srow{b}_{g}")
                        nc.vector.tensor_reduce(s_row[:], e_t[:], AX.X, ALU.add)
                        w_row = sm.tile([128, 1], F32, tag="wrow", bufs=16, name=f"wrow{b}_{g}")
                        nc.vector.reciprocal(w_row[:], s_row[:])
                        w_s = sm.tile([128, 1], F32, tag="ws", bufs=16, name=f"ws{b}_{g}")
                        nc.vector.tensor_scalar_mul(w_s[:], w_row[:], inv_s)
                        nc.vector.scalar_tensor_tensor(sc_acc[:], e_t[:], w_s[:],
                                                       sc_acc[:], ALU.mult, ALU.add)
                sc_accs[b] = sc_acc

            def fmv_extract(b, pool):
                # each fmv outputs 4 identical rows (ones lhsT with 4 cols):
                # row c of chunk n = the same column sums, bit-identical to a
                # [1,512] matvec, but staged on multiple partitions so
                # downstream DMAs read partitions in parallel
                # (single-partition SBUF reads are slow).
                s16 = sm.tile([16, 512], F32, tag="s16", bufs=2, name=f"s16_{b}")
                for n in range(4):
                    fmv = pool.tile([4, 512], F32, tag="fmv", bufs=2, name=f"fmv{b}_{n}")
                    nc.tensor.matmul(fmv[:], ones4[:],
                                     sc_accs[b][:, n * 512:(n + 1) * 512])
                    stage = sm.tile([4, 512], F32, tag="fmvs", bufs=4,
                                    name=f"fmvs{b}_{n}")
                    nc.vector.tensor_copy(stage[:], fmv[:])
                    nc.sync.dma_start(s16[4 * n:4 * (n + 1), :], stage[:])
                s_t = sm.tile([128, 16], F32, tag=f"st{b}", name=f"st{b}")
                for c in range(4):
                    nc.gpsimd.dma_start(
                        s_t[32 * c:32 * (c + 1), :],
                        s16[4 * c:4 * c + 1, :].rearrange("a (p i) -> a p i", p=32))
                s_ts[b] = s_t
                # spread into the radix layout: 4 quarter-copies per half
                for k in range(4):
                    nc.sync.dma_start(s128[b * 64 + 16 * k: b * 64 + 16 * (k + 1), :],
                                      s16[:])

            def radix_fused(psC):
                # exact v* (red-th largest) per batch via radix-16 search on
                # the positive-float bit ordering; both batches in one
                # [128, 512] layout. Counts are small-integer exact.
                t128 = sm.tile([128, 1], I32, tag="t128", bufs=2, name="t128")
                nc.vector.memset(t128[:], 0)
                for L in range(7, -1, -1):
                    cand = sm.tile([128, 1], I32, tag="cand", bufs=2,
                                   name=f"candf_{L}")
                    nc.vector.tensor_tensor(cand[:], t128[:], lvl128[L][:],
                                            ALU.bitwise_or)
                    cmp_t = sm.tile([128, 512], F32, tag="cmpf", bufs=1,
                                    name=f"cmpf_{L}")
                    cnt4 = sm.tile([128, 1], F32, tag="cnt4", bufs=2,
                                   name=f"cnt4_{L}")
                    nc.vector.tensor_scalar(cmp_t[:], s128[:],
                                            cand[:].bitcast(F32), 0.0,
                                            ALU.is_ge, ALU.add,
                                            accum_out=cnt4[:])
                    vm = sm.tile([128, 1], mybir.dt.float16, tag="vmf", bufs=2,
                                 name=f"vmf_{L}")
                    nc.vector.tensor_scalar(vm[:], cand[:], 0, None, ALU.is_ge)
                    cnt4h = sm.tile([128, 1], mybir.dt.float16, tag="cnt4h", bufs=2,
                                    name=f"cnt4h_{L}")
                    nc.vector.tensor_copy(cnt4h[:], cnt4[:])
                    cnt_ps = psC.tile([128, 1], F32, tag="rc", name=f"cntf_{L}")
                    nc.tensor.matmul(cnt_ps[:], Mj[:], cnt4h[:])
                    selj2 = sm.tile([128, 1], mybir.dt.float16, tag="selj2f", bufs=2,
                                    name=f"selj2f_{L}")
                    nc.vector.scalar_tensor_tensor(selj2[:], cnt_ps[:], float(red),
                                                   vm[:], ALU.is_ge, ALU.mult)
                    js_ps = psC.tile([128, 1], F32, tag="rc", name=f"jsf_{L}")
                    nc.tensor.matmul(js_ps[:], Mb[:], selj2[:])
                    jm1_i = sm.tile([128, 1], I32, tag="jm1fi", bufs=2,
                                    name=f"jm1fi_{L}")
                    nc.vector.tensor_scalar(jm1_i[:], js_ps[:], -1.0, None, ALU.add)
                    upd = sm.tile([128, 1], I32, tag="updf", bufs=2,
                                  name=f"updf_{L}")
                    nc.vector.tensor_scalar(upd[:], jm1_i[:], 4 * L, None,
                                            ALU.arith_shift_left)
                    t128n = sm.tile([128, 1], I32, tag="t128", bufs=2,
                                    name=f"t128n_{L}")
                    nc.vector.tensor_tensor(t128n[:], t128[:], upd[:],
                                            ALU.bitwise_or)
                    t128 = t128n
                # stage batch 1's threshold (partition 64) onto partition 0
                tb1s = sm.tile([1, 1], I32, tag="tb1s", name="tb1s")
                nc.sync.dma_start(tb1s[:], t128[64:65, 0:1])
                return t128, tb1s

            def phaseC_gen(b, psC, t128, tb1s):
                # post-threshold selection + gather; yields at cross-engine
                # dependency hops so two batches' chains interleave.
                s_t = s_ts[b]
                t_b = sm.tile([128, 1], F32, tag=f"tb{b}", name=f"tb{b}")
                if b == 0:
                    nc.gpsimd.partition_broadcast(t_b[:], t128[0:1, 0:1].bitcast(F32))
                else:
                    nc.gpsimd.partition_broadcast(t_b[:], tb1s[0:1, 0:1].bitcast(F32))
                yield
                # cnt_gt and m
                sel0 = sm.tile([128, 16], F32, tag=f"sel0{b}", name=f"sel0{b}")
                rs_sel = sm.tile([128, 1], F32, tag=f"rssel{b}", name=f"rssel{b}")
                nc.vector.tensor_scalar(sel0[:], s_t[:], t_b[:], 0.0, ALU.is_gt,
                                        ALU.add, accum_out=rs_sel[:])
                cnt_ps = psC.tile([1, 1], F32, tag=f"c{b}", name=f"cnt{b}")
                nc.tensor.matmul(cnt_ps[:], ones_t[:], rs_sel[:])
                yield
                m_t = sm.tile([1, 1], F32, tag=f"mt{b}", name=f"mt{b}")
                nc.vector.tensor_scalar(m_t[:], cnt_ps[:], -1.0, float(red),
                                        ALU.mult, ALU.add)
                m_b = sm.tile([128, 1], F32, tag=f"mb{b}", name=f"mb{b}")
                nc.gpsimd.partition_broadcast(m_b[:], m_t[:])
                tie = sm.tile([128, 16], F32, tag=f"tie{b}", name=f"tie{b}")
                nc.vector.tensor_scalar(tie[:], s_t[:], t_b[:], None, ALU.is_equal)
                scan_tie = sm.tile([128, 16], F32, tag=f"scant{b}", name=f"scant{b}")
                nc.vector.tensor_tensor_scan(scan_tie[:], tie[:], zz16[:], 0.0,
                                             ALU.add, ALU.add)
                rs_tie = sm.tile([128, 1], F32, tag=f"rstie{b}", name=f"rstie{b}")
                nc.vector.tensor_reduce(rs_tie[:], tie[:], AX.X, ALU.add)
                offt_ps = psC.tile([128, 1], F32, tag=f"c{b}", name=f"offt{b}")
                nc.tensor.matmul(offt_ps[:], u_strict[:], rs_tie[:])
                yield
                off_tie = sm.tile([128, 1], F32, tag=f"offtie{b}", name=f"offtie{b}")
                nc.vector.tensor_copy(off_tie[:], offt_ps[:])
                p_tie = sm.tile([128, 16], F32, tag=f"ptie{b}", name=f"ptie{b}")
                nc.vector.tensor_scalar(p_tie[:], scan_tie[:], off_tie[:], None, ALU.add)

                cond = sm.tile([128, 16], F32, tag=f"cond{b}", name=f"cond{b}")
                nc.vector.tensor_scalar(cond[:], p_tie[:], m_b[:], None, ALU.is_le)
                tsel = sm.tile([128, 16], F32, tag=f"tsel{b}", name=f"tsel{b}")
                nc.vector.tensor_mul(tsel[:], tie[:], cond[:])
                mask = sm.tile([128, 16], F32, tag=f"mask{b}", name=f"mask{b}")
                nc.vector.tensor_add(mask[:], sel0[:], tsel[:])

                scan_m = sm.tile([128, 16], F32, tag=f"scanm{b}", name=f"scanm{b}")
                nc.vector.tensor_tensor_scan(scan_m[:], mask[:], zz16[:], 0.0,
                                             ALU.add, ALU.add)
                rs_m = sm.tile([128, 1], F32, tag=f"rsm{b}", name=f"rsm{b}")
                nc.vector.tensor_reduce(rs_m[:], mask[:], AX.X, ALU.add)
                offm_ps = psC.tile([128, 1], F32, tag=f"c{b}", name=f"offm{b}")
                nc.tensor.matmul(offm_ps[:], u_strict[:], rs_m[:])
                yield
                off_m = sm.tile([128, 1], F32, tag=f"offm{b}", name=f"offmsb{b}")
                nc.vector.tensor_copy(off_m[:], offm_ps[:])
                csum = sm.tile([128, 16], F32, tag=f"csum{b}", name=f"csum{b}")
                nc.vector.tensor_scalar(csum[:], scan_m[:], off_m[:], None, ALU.add)

                # pos0 = mask*(csum+15) - 16  (selected: 0..red-1; unselected: -16)
                t1 = sm.tile([128, 16], F32, tag=f"t1{b}", name=f"t1{b}")
                nc.vector.tensor_scalar(t1[:], csum[:], 15.0, None, ALU.add)
                p1 = sm.tile([128, 16], F32, tag=f"p1{b}", name=f"p1{b}")
                nc.vector.tensor_mul(p1[:], t1[:], mask[:])
                pos0 = sm.tile([128, 16], F32, tag=f"pos0{b}", name=f"pos0{b}")
                nc.vector.tensor_scalar(pos0[:], p1[:], -16.0, None, ALU.add)

                pos_i = sm.tile([128, 16], I32, tag=f"posi{b}", name=f"posi{b}")
                nc.vector.tensor_copy(pos_i[:], pos0[:])
                f_i = sm.tile([128, 16], I32, tag=f"fi{b}", name=f"fi{b}")
                nc.vector.tensor_scalar(f_i[:], pos_i[:], 4, None, ALU.arith_shift_right)
                f16_i = sm.tile([128, 16], I32, tag=f"f16i{b}", name=f"f16i{b}")
                nc.vector.tensor_scalar(f16_i[:], f_i[:], 4, None, ALU.arith_shift_left)
                w_i = sm.tile([128, 16], I32, tag=f"wi{b}", name=f"wi{b}")
                nc.vector.tensor_sub(w_i[:], pos_i[:], f16_i[:])
                f_f = sm.tile([128, 16], F32, tag=f"ff{b}", name=f"ff{b}")
                nc.vector.tensor_copy(f_f[:], f_i[:])
                w_f = sm.tile([128, 16], F32, tag=f"wf{b}", name=f"wf{b}")
                nc.vector.tensor_copy(w_f[:], w_i[:])
                yield

                idx_ps = psC.tile([128, nslots], F32, tag=f"c{b}", name=f"idxps{b}")
                for i in range(16):
                    a_i = sm.tile([128, 128], mybir.dt.float16, tag=f"ai{b}",
                                  name=f"ai{b}_{i}")
                    nc.vector.tensor_scalar(a_i[:], colm16[:], w_f[:, i:i + 1],
                                            jcol_f[:, i:i + 1], ALU.is_equal, ALU.mult)
                    b_i = sm.tile([128, nslots], mybir.dt.float16, tag=f"bi{b}",
                                  name=f"bi{b}_{i}")
                    nc.vector.tensor_scalar(b_i[:], iota32[:], f_f[:, i:i + 1], None,
                                            ALU.is_equal)
                    nc.tensor.matmul(idx_ps[:], a_i[:], b_i[:],
                                     start=(i == 0), stop=(i == 15))
                    if i % 6 == 5:
                        yield
                yield

                idx_f = sm.tile([128, nslots], F32, tag=f"idxf{b}", name=f"idxf{b}")
                nc.vector.tensor_scalar(idx_f[:], idx_ps[:], 1.0, None, ALU.add)
                idx_pm = sm.tile([128, nslots], F32, tag=f"idxpm{b}", name=f"idxpm{b}")
                nc.vector.tensor_mul(idx_pm[:], idx_f[:], padmask[:])
                idx_fin = sm.tile([128, nslots], F32, tag=f"idxfin{b}", name=f"idxfin{b}")
                nc.vector.tensor_scalar(idx_fin[:], idx_pm[:], -1.0, None, ALU.add)
                idx128 = sm.tile([128, nslots], I16, tag=f"idx128{b}", name=f"idx128{b}")
                nc.vector.tensor_copy(idx128[:], idx_fin[:])
                yield

                gath = sm.tile([128, npad // 128, D], F32, tag=f"gath{b}", name=f"gath{b}")
                half = npad // 2                      # 256
                hs = half // 16                       # 16 idx slots per half
                hc = half // 128                      # 2 row-groups per half
                nc.gpsimd.dma_gather(gath[:, 0:hc, :], x_ext[b][:],
                                     idx128[:, 0:hs], num_idxs=half,
                                     num_idxs_reg=half, elem_size=D)
                nc.sync.dma_start(
                    out_ext[b, 0:half, :].rearrange("(c p) d -> p c d", c=hc),
                    gath[:, 0:hc, :])
                yield
                nc.gpsimd.dma_gather(gath[:, hc:2 * hc, :], x_ext[b][:],
                                     idx128[:, hs:2 * hs], num_idxs=half,
                                     num_idxs_reg=red - half, elem_size=D)
                if nfull > hc:
                    nc.sync.dma_start(
                        out_ext[b, half:nfull * 128, :].rearrange(
                            "(c p) d -> p c d", c=nfull - hc),
                        gath[:, hc:nfull, :])
                if ntail:
                    nc.sync.dma_start(out_ext[b, nfull * 128:red, :],
                                      gath[0:ntail, nfull, :])

            s_ts = {}
            phaseA(0)
            phaseB(0)
            phaseA(1)
            with tc.tile_pool(name="psF0", bufs=1, space=PSUM) as psF0:
                fmv_extract(0, psF0)
            phaseB(1)

            with tc.tile_pool(name="psC", bufs=2, space=PSUM) as psC:
                fmv_extract(1, psC)
                t128, tb1s = radix_fused(psC)
                gens = [phaseC_gen(b, psC, t128, tb1s) for b in range(BPC)]
                done = [False] * BPC
                while not all(done):
                    for i, g in enumerate(gens):
                        if not done[i]:
                            try:
                                next(g)
                            except StopIteration:
                                done[i] = True

    # schedule audit: for every PSUM tile, its matmuls must appear in the
    # emitted stream (a) start-first and (b) in program order (instruction
    # ids are monotonically assigned at trace time), so fp32 accumulation
    # order is deterministic. The Tile scheduler is nondeterministic; a bad
    # draw is caught here (the caller rebuilds).
    first_mm = {}
    last_id = {}
    ok = True
    for blk in nc.main_func.blocks:
        for ins in blk.instructions:
            if isinstance(ins, mybir.InstMatmult):
                out = ins.outs[0]
                mloc = getattr(out, "memory_location", None)
                name = mloc.name if mloc is not None else getattr(out, "memref", str(out))
                try:
                    iid = int(str(ins.name).split("-")[-1])
                except ValueError:
                    iid = None
                if name not in first_mm:
                    first_mm[name] = ins.start_tensor_calc
                    if not ins.start_tensor_calc:
                        ok = False
                if iid is not None:
                    if name in last_id and iid < last_id[name]:
                        ok = False
                    last_id[name] = iid
    if not ok:
        return None
    nc.compile()
    return nc


_CACHE = {}


def kernel(**inputs):
    from concourse.bass_utils import run_bass_kernel_spmd

    x = np.ascontiguousarray(np.asarray(inputs["x"], dtype=np.float32))
    Wq = np.asarray(inputs["Wq"], dtype=np.float32)
    Wk = np.asarray(inputs["Wk"], dtype=np.float32)
    bq = np.asarray(inputs["bq"], dtype=np.float32)
    bk = np.asarray(inputs["bk"], dtype=np.float32)
    temp = float(np.asarray(inputs["temperature"], dtype=np.float32).reshape(-1)[0])
    num_tokens = int(np.asarray(inputs["num_tokens"]))
    red = int(num_tokens * 0.2)

    key = (red, np.float32(temp).tobytes())
    if key not in _CACHE:
        built = None
        for _attempt in range(4):
            built = _build(red, temp)
            if built is not None:
                break
        assert built is not None, "scheduler audit failed on 4 consecutive builds"
        _CACHE[key] = built
    nc = _CACHE[key]

    wqT = np.ascontiguousarray(Wq.T)  # [D, R]
    wkT = np.ascontiguousarray(Wk.T)
    xT = np.ascontiguousarray(np.swapaxes(x, 1, 2))  # [B, D, S]
    in_maps = [
        {"x": x[i * BPC:(i + 1) * BPC], "xT": xT[i * BPC:(i + 1) * BPC],
         "wqT": wqT, "wkT": wkT, "bq": bq, "bk": bk}
        for i in range(N_CORES)
    ]
    trace = bool(int(os.environ.get("ATRM_TRACE", "0")))
    res = run_bass_kernel_spmd(nc, in_maps, list(range(N_CORES)), trace=trace)
    kernel.last_result = res
    out = np.concatenate([r["out"] for r in res.results], axis=0)
    return out.astype(np.float32)
